# revision 1
# baseline (speedup 1.0000x reference)
"""Trainium2 Bass kernel for CrossAttention (B=4, T=2048, S=4096, D=256, H=8, Dh=32).

Sharding: 8 cores = 4 batches x 2 T-halves (each core owns 1024 query rows of
one batch, all heads). No collectives needed: each core computes its full
output rows; host concatenates.

Per-core dataflow (all "T"-like dims on the free axis, contractions on
partitions):
  xT [256, 1024], cT [256, 4096] via PE transposes (fp32 in, fp16 out)
  qT = w_q^T @ xT     [256(hid), 1024]
  kT = w_k^T @ cT     [256(hid), 4096]
  v  = cT^T @ w_v     [4096(S), 256] stored interleaved with a ones column
                      per head (v' [S, 33] per head) so attn@v' also yields
                      the softmax denominator for free.
  per (head, S-tile of 128):
    sT = kT_h_tile^T @ qT_h   [128(S), 1024(T)]  (fp16 matmul -> PSUM fp32)
    attnT = exp(sT * scale)   (ScalarE, PSUM->SBUF fp16)
    outT'_h += v'_h_tile^T @ attnT   [33, 1024] accumulated in PSUM fp32
  normalize outT by the broadcast reciprocal denominator,
  out = outT^T @ w_out + b_out.

Matmuls use fp16 operands (1 cycle/row; fp32 is 4x) with fp32 PSUM
accumulation. The structure is shaped by a hardware constraint: a PE
instruction can carry only ONE semaphore wait, so every matmul is arranged
to depend on at most one other engine (one shared PSUM pool, accumulator
dumps on ScalarE so slot releases merge with the exp waits, etc).
"""

import sys

if "/opt/trn_rl_repo" not in sys.path:
    sys.path.insert(0, "/opt/trn_rl_repo")

from contextlib import ExitStack

import numpy as np

import concourse.bass as bass
import concourse.tile as tile
from concourse import bacc
from concourse import mybir
from concourse.bass_utils import run_bass_kernel_spmd

B, T, S, D, H, Dh = 4, 2048, 4096, 256, 8, 32
TL = T // 2  # 1024 query rows per core
NXT = TL // 128  # 8 x tiles
SCALE = Dh ** -0.5
FP = mybir.dt.float32
F16 = mybir.dt.float16
NST = S // 128  # 32 S-tiles
VW = H * (Dh + 1)  # 264 packed v' columns per S-tile
# head h -> (triple tile, 32-row block): heads grouped 3+3+2 so every row
# block starts at partition 0/32/64 (hardware base-partition constraint)
TRIP = [(h // 3, h % 3) for h in range(H)]
TRIP_HEADS = [[0, 1, 2], [3, 4, 5], [6, 7]]


def build_bass():
    nc = bacc.Bacc()
    ident_d = nc.declare_dram_parameter("ident", [128, 128], FP, isOutput=False)
    x_d = nc.declare_dram_parameter("x", [TL, D], FP, isOutput=False)
    ctx_d = nc.declare_dram_parameter("context", [S, D], FP, isOutput=False)
    wq_d = nc.declare_dram_parameter("w_q", [D, D], FP, isOutput=False)
    wkv_d = nc.declare_dram_parameter("w_kv", [D, 2 * D], FP, isOutput=False)
    wout_d = nc.declare_dram_parameter("w_out", [D, D], FP, isOutput=False)
    bout_d = nc.declare_dram_parameter("b_out", [1, D], FP, isOutput=False)
    out_d = nc.declare_dram_parameter("out", [TL, D], FP, isOutput=True)
    dnscr = nc.dram_tensor("dnscratch", [H, TL], FP)

    with tile.TileContext(nc) as tc, ExitStack() as ctx:
        consts = ctx.enter_context(tc.tile_pool(name="consts", bufs=1))
        persist = ctx.enter_context(tc.tile_pool(name="persist", bufs=1))
        psum = ctx.enter_context(tc.tile_pool(name="psum", bufs=3, space="PSUM"))

        identity = consts.tile([128, 128], FP, tag="identity", name="identity")
        # hsel[b] [1, 96]: ones in columns 32b..32b+32 — builds the per-head
        # reciprocal broadcast via K=1 accumulating matmuls
        hsel = []
        for b in range(3):
            m = consts.tile([1, 96], F16, tag=f"hsel{b}", name=f"hsel{b}")
            nc.vector.memset(m, 0.0)
            nc.vector.memset(m[0:1, 32 * b : 32 * b + 32], 1.0)
            hsel.append(m)

        wq = [persist.tile([128, D], F16, tag=f"wq{j}", name=f"wq{j}") for j in range(2)]
        wkv = [persist.tile([128, 2 * D], F16, tag=f"wkv{j}", name=f"wkv{j}") for j in range(2)]
        wo_rows = [96, 96, 64]
        woutg = [
            persist.tile([wo_rows[t], D], F16, tag=f"woutg{t}", name=f"woutg{t}")
            for t in range(3)
        ]
        bias_b = persist.tile([128, D], FP, tag="bias_b", name="bias_b")
        bias_c = persist.tile([128, D], FP, tag="bias_c", name="bias_c")
        # 2 heads per tile (base-partition constraint)
        qT = [persist.tile([64, TL], F16, tag=f"qT{j}", name=f"qT{j}") for j in range(4)]
        kT = [persist.tile([64, S], F16, tag=f"kT{j}", name=f"kT{j}") for j in range(4)]
        vP = persist.tile([128, NST * VW], F16, tag="vP", name="vP")
        dumpT = [
            persist.tile([96, TL], F16, tag=f"dumpT{t}", name=f"dumpT{t}")
            for t in range(3)
        ]
        outTh = [
            persist.tile([96, TL], F16, tag=f"outTh{t}", name=f"outTh{t}")
            for t in range(3)
        ]
        rcp = [persist.tile([96, TL], FP, tag=f"rcp{t}", name=f"rcp{t}") for t in range(3)]
        rden = [persist.tile([96, TL], FP, tag=f"rden{t}", name=f"rden{t}") for t in range(3)]
        xT = [persist.tile([128, TL], F16, tag=f"xT{j}", name=f"xT{j}") for j in range(2)]
        cT = [persist.tile([128, S], F16, tag=f"cT{j}", name=f"cT{j}") for j in range(2)]
        early = tc.tile_pool(name="early", bufs=1)
        ep = early.__enter__()
        x_all = ep.tile([128, NXT, D], FP, tag="x_all", name="x_all")
        c_all = ep.tile([128, NST, D], FP, tag="c_all", name="c_all")
        wstage = [
            ep.tile([128, 3 * D], FP, tag=f"wstage{j}", name=f"wstage{j}")
            for j in range(2)
        ]
        wso = [
            ep.tile([wo_rows[t], D], FP, tag=f"wso{t}", name=f"wso{t}")
            for t in range(3)
        ]

        # ---- Phase 0: loads + fp16 weight conversion ----
        # DMA issue order is load-bearing: the HW DGE queue is assigned
        # round-robin (mod 8) over DMA program order. ident is DMA #0 and
        # x_all #8 (same queue), c_all #9 -> the two first-touch transposes
        # each carry exactly one queue wait, and PE never needs a second one.
        nc.sync.dma_start(out=identity, in_=ident_d[:, :])
        for j in range(2):
            nc.sync.dma_start(out=wstage[j][:, 0:D], in_=wq_d[128 * j : 128 * j + 128, :])
            nc.sync.dma_start(
                out=wstage[j][:, D : 3 * D], in_=wkv_d[128 * j : 128 * j + 128, :]
            )
            nc.vector.tensor_copy(wq[j], wstage[j][:, 0:D])
            nc.vector.tensor_copy(wkv[j], wstage[j][:, D : 3 * D])
        ro = 0
        for t in range(3):
            nc.sync.dma_start(out=wso[t], in_=wout_d[ro : ro + wo_rows[t], :])
            nc.vector.tensor_copy(woutg[t], wso[t])
            ro += wo_rows[t]
        nc.sync.dma_start(out=x_all, in_=x_d.rearrange("(t p) d -> p t d", p=128))
        ctx_r = ctx_d.rearrange("(t p) d -> p t d", p=128)
        for cc in range(4):
            nc.sync.dma_start(out=c_all[:, 8 * cc : 8 * cc + 8, :], in_=ctx_r[:, 8 * cc : 8 * cc + 8, :])
        nc.sync.dma_start(out=bias_b, in_=bout_d[0:1, :].partition_broadcast(128))
        nc.vector.tensor_copy(bias_c, bias_b)

        # ---- Phase 1: transpose x and context ----
        # convert to fp16 first: fp32 PE transposes run 4 cycles/row
        xh = persist.tile([128, NXT, D], F16, tag="xh", name="xh")
        ch = persist.tile([128, NST, D], F16, tag="ch", name="ch")
        idh = consts.tile([128, 128], F16, tag="idh", name="idh")
        nc.vector.tensor_copy(idh, identity)
        nc.vector.tensor_copy(xh, x_all)
        for cc in range(4):
            nc.vector.tensor_copy(ch[:, 8 * cc : 8 * cc + 8, :], c_all[:, 8 * cc : 8 * cc + 8, :])

        tcount = [0]

        def do_transpose(src_all, st, j, dstT):
            pt = psum.tile([128, 128], F16, tag="sc", name="pt")
            nc.tensor.transpose(pt, src_all[:, st, 128 * j : 128 * j + 128], idh)
            if tcount[0] % 2 == 0:
                nc.vector.tensor_copy(dstT[:, 128 * st : 128 * st + 128], pt)
            else:
                nc.scalar.copy(dstT[:, 128 * st : 128 * st + 128], pt)
            tcount[0] += 1

        for t in range(NXT):
            for j in range(2):
                do_transpose(xh, t, j, xT[j])
        for st in range(NST):
            for j in range(2):
                do_transpose(ch, st, j, cT[j])

        early.__exit__(None, None, None)
        attnp = ctx.enter_context(tc.tile_pool(name="attn", bufs=16))
        dnp = ctx.enter_context(tc.tile_pool(name="dnp", bufs=2))
        fstage = ctx.enter_context(tc.tile_pool(name="fstage", bufs=4))

        # ---- Phase 2: projections ----
        def qk_proj(mj):
            for nt in range(TL // 512):
                pq = psum.tile([128, 512], FP, tag="sc", name="pq")
                for kj in range(2):
                    nc.tensor.matmul(
                        pq,
                        lhsT=wq[kj][:, 128 * mj : 128 * mj + 128],
                        rhs=xT[kj][:, 512 * nt : 512 * nt + 512],
                        start=(kj == 0),
                        stop=(kj == 1),
                    )
                for half in range(2):
                    nc.vector.tensor_copy(
                        qT[2 * mj + half][:, 512 * nt : 512 * nt + 512],
                        pq[64 * half : 64 * half + 64, :],
                    )
            for nt in range(S // 512):
                pk = psum.tile([128, 512], FP, tag="sc", name="pk")
                for kj in range(2):
                    nc.tensor.matmul(
                        pk,
                        lhsT=wkv[kj][:, 128 * mj : 128 * mj + 128],
                        rhs=cT[kj][:, 512 * nt : 512 * nt + 512],
                        start=(kj == 0),
                        stop=(kj == 1),
                    )
                nc.vector.tensor_copy(
                    kT[2 * mj][:, 512 * nt : 512 * nt + 512], pk[0:64, :]
                )
                nc.scalar.copy(
                    kT[2 * mj + 1][:, 512 * nt : 512 * nt + 512], pk[64:128, :]
                )

        def v_proj(st_lo, st_hi):
            for st in range(st_lo, st_hi):
                pv = psum.tile([128, D], FP, tag="sc", name="pv")
                for kj in range(2):
                    nc.tensor.matmul(
                        pv,
                        lhsT=cT[kj][:, 128 * st : 128 * st + 128],
                        rhs=wkv[kj][:, D : 2 * D],
                        start=(kj == 0),
                        stop=(kj == 1),
                    )
                dst = vP[:, VW * st : VW * st + VW].rearrange(
                    "p (h w) -> p h w", h=H
                )[:, :, 0:Dh]
                nc.vector.tensor_copy(dst, pv.rearrange("p (h w) -> p h w", h=H))

        ones_cols = vP.rearrange("p (s h w) -> p s h w", s=NST, h=H)[:, :, :, Dh : Dh + 1]
        nc.vector.memset(ones_cols, 1.0)
        qk_proj(0)
        v_proj(0, 8)
        qk_proj(1)
        v_proj(8, NST)

        # ---- Phase 3: fused attention ----
        # S-tiles processed in pairs: grouping the K=32 score matmuls and the
        # K=128 attn@v matmuls into runs halves the PE K-geometry switches
        # (~200ns each)
        dn_tiles = []
        for h in range(H):
            jj, aa = h // 2, h % 2
            tt_, bb_ = TRIP[h]
            acc = psum.tile([Dh + 1, TL], FP, tag="acc", name="acc", bufs=1)
            GRP = 4
            for sp in range(NST // GRP):
                sts = range(GRP * sp, GRP * sp + GRP)
                scs = []
                for st in sts:
                    sc = psum.tile([128, TL], FP, tag="sc", name="sc")
                    for nt in range(2):
                        nc.tensor.matmul(
                            sc[:, 512 * nt : 512 * nt + 512],
                            lhsT=kT[jj][32 * aa : 32 * aa + 32, 128 * st : 128 * st + 128],
                            rhs=qT[jj][32 * aa : 32 * aa + 32, 512 * nt : 512 * nt + 512],
                            start=True,
                            stop=True,
                            skip_group_check=True,
                        )
                    scs.append(sc)
                ats = []
                for sc in scs:
                    at = attnp.tile([128, TL], F16, tag="at", name="at")
                    nc.scalar.activation(
                        at, sc, mybir.ActivationFunctionType.Exp, scale=SCALE
                    )
                    ats.append(at)
                for i, st in enumerate(sts):
                    at = ats[i]
                    for nt in range(2):
                        nc.tensor.matmul(
                            acc[:, 512 * nt : 512 * nt + 512],
                            lhsT=vP[:, VW * st + (Dh + 1) * h : VW * st + (Dh + 1) * h + Dh + 1],
                            rhs=at[:, 512 * nt : 512 * nt + 512],
                            start=(st == 0),
                            stop=(st == NST - 1),
                            skip_group_check=True,
                        )
            nc.vector.tensor_copy(dumpT[tt_][32 * bb_ : 32 * bb_ + 32, :], acc[0:Dh, :])
            dnt = dnp.tile([1, TL], FP, tag="dn", name="dn")
            nc.vector.tensor_copy(dnt, acc[Dh : Dh + 1, :])
            dn_tiles.append(dnt)
            nc.sync.dma_start(out=dnscr[h : h + 1, :], in_=dnt)
            if bb_ == len(TRIP_HEADS[tt_]) - 1:
                t = tt_
                heads = TRIP_HEADS[t]
                nr = 32 * len(heads)
                for bi, hh in enumerate(heads):
                    nc.sync.dma_start(
                        out=rden[t][32 * bi : 32 * bi + 32, :],
                        in_=dnscr[hh : hh + 1, :].partition_broadcast(32),
                    )
                nc.vector.reciprocal_approx_fast(rcp[t][0:nr, :], rden[t][0:nr, :])
                for bi in range(len(heads)):
                    nc.vector.tensor_mul(
                        outTh[t][32 * bi : 32 * bi + 32, :],
                        dumpT[t][32 * bi : 32 * bi + 32, :],
                        rcp[t][32 * bi : 32 * bi + 32, :],
                    )

        # ---- Phase 4: output projection (normalization ran in phase 3) ----
        for tt in range(TL // 128):
            fin = psum.tile([128, D], FP, tag="sc", name="fin")
            for t in range(3):
                nc.tensor.matmul(
                    fin,
                    lhsT=outTh[t][0 : wo_rows[t], 128 * tt : 128 * tt + 128],
                    rhs=woutg[t],
                    start=(t == 0),
                    stop=(t == 2),
                )
            outs = fstage.tile([128, D], FP, tag="outs", name="outs")
            nc.vector.tensor_add(outs, fin, bias_c)
            nc.sync.dma_start(out=out_d[128 * tt : 128 * tt + 128, :], in_=outs)

    nc.compile()
    return nc


_NC = None


def kernel(**inputs):
    global _NC
    x = np.ascontiguousarray(inputs["x"], dtype=np.float32)
    context = np.ascontiguousarray(inputs["context"], dtype=np.float32)
    w_q = np.ascontiguousarray(inputs["w_q"], dtype=np.float32)
    w_kv = np.ascontiguousarray(inputs["w_kv"], dtype=np.float32)
    w_out = np.ascontiguousarray(inputs["w_out"], dtype=np.float32)
    b_out = np.ascontiguousarray(inputs["b_out"], dtype=np.float32).reshape(1, D)

    if _NC is None:
        _NC = build_bass()
    nc = _NC

    in_maps = []
    for c in range(8):
        b, half = c // 2, c % 2
        in_maps.append(
            {
                "ident": np.eye(128, dtype=np.float32),
                "x": np.ascontiguousarray(x[b, TL * half : TL * half + TL, :]),
                "context": np.ascontiguousarray(context[b]),
                "w_q": w_q,
                "w_kv": w_kv,
                "w_out": w_out,
                "b_out": b_out,
            }
        )
    res = run_bass_kernel_spmd(nc, in_maps, core_ids=list(range(8)))
    out = np.empty((B, T, D), dtype=np.float32)
    for c in range(8):
        b, half = c // 2, c % 2
        out[b, TL * half : TL * half + TL, :] = res.results[c]["out"]
    return out


if __name__ == "__main__":
    rng = np.random.default_rng(0)
    ins = {
        "x": rng.standard_normal((B, T, D), dtype=np.float32),
        "context": rng.standard_normal((B, S, D), dtype=np.float32),
        "w_q": rng.standard_normal((D, D), dtype=np.float32) * D**-0.5,
        "w_kv": rng.standard_normal((D, 2 * D), dtype=np.float32) * D**-0.5,
        "w_out": rng.standard_normal((D, D), dtype=np.float32) * D**-0.5,
        "b_out": rng.standard_normal((D,), dtype=np.float32) * 0.01,
    }
    out = kernel(**ins)
    print(out.shape, out.dtype, np.abs(out).mean())



# revision 9
# speedup vs baseline: 1.3145x; 1.3145x over previous
"""Trainium2 Bass kernel for CrossAttention (B=4, T=2048, S=4096, D=256, H=8, Dh=32).

Sharding: 8 cores = 4 batches x 2 T-halves (each core owns 1024 query rows of
one batch, all heads). No collectives: host concatenates.

v2 design (vs baseline):
- Scores are 4-way PE row-tiled: qT/kT packed per 4-head group across the 128
  partitions (head h at partitions 32h..32h+32); the 4 K=32 score matmuls of a
  chunk are issued back-to-back at tile_position (0,0)/(32,0)/(64,0)/(96,0) and
  run concurrently on the 4 row-strips of the PE array (~3x).
- attn@v uses M=33 (32 v dims + ones column for the softmax denominator) with
  2-way column tiling: heads paired at tile_position (0,0)/(0,64), accumulating
  over all 32 S-chunks into two persistent PSUM accumulator banks.
- The 33.5M softmax exps per core are split ~50/50 between ScalarE (exact LUT
  activation) and the Vector engine using a Schraudolph bit-trick exp:
  int16(x*a+b) interpreted as fp16 IS exp(x) to ~2-3% element accuracy, which
  the softmax normalization averages down to ~5e-3 output error. The DVE path
  is two ops: PSUM->SBUF fp16 copy (2x mode) then fp16->int16 tensor_scalar
  (4x mode) written through a bitcast into the fp16 at tile.
- PSUM budget: 3 rotating score slots of [128,1024] (one head-pair each,
  2 banks) + 2 attn@v accumulators = exactly 8 banks.
"""

import sys

if "/opt/trn_rl_repo" not in sys.path:
    sys.path.insert(0, "/opt/trn_rl_repo")

from contextlib import ExitStack

import numpy as np

import concourse.bass as bass
import concourse.tile as tile
from concourse import bacc
from concourse import mybir
from concourse.bass_utils import run_bass_kernel_spmd

B, T, S, D, H, Dh = 4, 2048, 4096, 256, 8, 32
TL = T // 2          # 1024 query rows per core
NST = S // 128       # 32 S-chunks
SCALE = Dh ** -0.5
FP = mybir.dt.float32
F16 = mybir.dt.float16
I16 = mybir.dt.int16

LOG2E = 1.4426950408889634
SCHR_A = SCALE * 1024.0 * LOG2E     # exp(SCALE*x) ~ fp16bits(int16(x*A + B))
SCHR_B = 15360.0 - 14.8


def build_bass():
    nc = bacc.Bacc()
    ident_d = nc.declare_dram_parameter("ident", [128, 128], FP, isOutput=False)
    x_d = nc.declare_dram_parameter("x", [TL, D], FP, isOutput=False)
    ctx_d = nc.declare_dram_parameter("context", [S, D], FP, isOutput=False)
    wq_d = nc.declare_dram_parameter("w_q", [D, D], FP, isOutput=False)
    wkv_d = nc.declare_dram_parameter("w_kv", [D, 2 * D], FP, isOutput=False)
    wout_d = nc.declare_dram_parameter("w_out", [D, D], FP, isOutput=False)
    bout_d = nc.declare_dram_parameter("b_out", [1, D], FP, isOutput=False)
    out_d = nc.declare_dram_parameter("out", [TL, D], FP, isOutput=True)
    dnscr = nc.dram_tensor("dnscratch", [H, TL], FP)

    with tile.TileContext(nc) as tc, ExitStack() as ctx:
        consts = ctx.enter_context(tc.tile_pool(name="consts", bufs=1))
        persist = ctx.enter_context(tc.tile_pool(name="persist", bufs=1))

        idh = consts.tile([128, 128], F16, tag="idh", name="idh")
        bias_c = persist.tile([128, D], FP, tag="bias_c", name="bias_c")

        # fp16 weights
        wqh = [persist.tile([128, D], F16, tag=f"wqh{j}", name=f"wqh{j}") for j in range(2)]
        wkh = [persist.tile([128, D], F16, tag=f"wkh{j}", name=f"wkh{j}") for j in range(2)]
        wvh = [persist.tile([128, D], F16, tag=f"wvh{j}", name=f"wvh{j}") for j in range(2)]
        woh = [persist.tile([128, D], F16, tag=f"woh{j}", name=f"woh{j}") for j in range(2)]

        # transposed activations (d on partitions)
        xT = [persist.tile([128, TL], F16, tag=f"xT{j}", name=f"xT{j}") for j in range(2)]
        cT = [persist.tile([128, S], F16, tag=f"cT{j}", name=f"cT{j}") for j in range(2)]
        # packed projections: group g holds heads 4g..4g+3, head j at partitions 32j..
        qT = [persist.tile([128, TL], F16, tag=f"qT{g}", name=f"qT{g}") for g in range(2)]
        kT = [persist.tile([128, S], F16, tag=f"kT{g}", name=f"kT{g}") for g in range(2)]
        # v packed per chunk/head: [s=128, chunk, head, 34] (32 v dims + ones + pad)
        vP = persist.tile([128, NST, H, 34], F16, tag="vP", name="vP")
        # normalized attention output, lhsT layout for the out projection
        outN = [persist.tile([128, TL], F16, tag=f"outN{g}", name=f"outN{g}") for g in range(2)]

        # ---------------- Phase A: load + convert + transpose + project ----
        ea = tc.tile_pool(name="early", bufs=1)
        ep = ea.__enter__()
        eps = tc.tile_pool(name="early_ps", bufs=2, space="PSUM")
        epp = eps.__enter__()

        ident_s = ep.tile([128, 128], FP, tag="ident_s", name="ident_s")
        x_all = ep.tile([128, TL // 128, D], FP, tag="x_all", name="x_all")
        c_all = ep.tile([128, NST, D], FP, tag="c_all", name="c_all")
        wstage = ep.tile([128, 6 * D], FP, tag="wstage", name="wstage")
        xh = ep.tile([128, TL // 128, D], F16, tag="xh", name="xh")
        ch = ep.tile([128, NST, D], F16, tag="ch", name="ch")

        nc.sync.dma_start(out=ident_s, in_=ident_d[:, :])
        for j in range(2):
            nc.sync.dma_start(
                out=wstage[:, j * D : j * D + D], in_=wq_d[128 * j : 128 * j + 128, :]
            )
            nc.sync.dma_start(
                out=wstage[:, (2 + 2 * j) * D : (4 + 2 * j) * D],
                in_=wkv_d[128 * j : 128 * j + 128, :],
            )
        nc.sync.dma_start(out=x_all, in_=x_d.rearrange("(t p) d -> p t d", p=128))
        ctx_r = ctx_d.rearrange("(t p) d -> p t d", p=128)
        for cc in range(4):
            nc.sync.dma_start(
                out=c_all[:, 8 * cc : 8 * cc + 8, :], in_=ctx_r[:, 8 * cc : 8 * cc + 8, :]
            )
        nc.sync.dma_start(out=bias_c, in_=bout_d[0:1, :].partition_broadcast(128))

        nc.vector.tensor_copy(idh, ident_s)
        for j in range(2):
            nc.scalar.copy(wqh[j], wstage[:, j * D : j * D + D])
            nc.scalar.copy(wkh[j], wstage[:, (2 + 2 * j) * D : (3 + 2 * j) * D])
            nc.scalar.copy(wvh[j], wstage[:, (3 + 2 * j) * D : (4 + 2 * j) * D])
        # w_out rows 128g.. as rhs tiles [hid-part, dout]
        wos = ep.tile([128, 2, D], FP, tag="wos", name="wos")
        for g in range(2):
            nc.sync.dma_start(out=wos[:, g, :], in_=wout_d[128 * g : 128 * g + 128, :])
            nc.vector.tensor_copy(woh[g], wos[:, g, :])

        nc.vector.tensor_copy(xh, x_all)
        for cc in range(4):
            blk = ch[:, 8 * cc : 8 * cc + 8, :]
            if cc % 2 == 0:
                nc.vector.tensor_copy(blk, c_all[:, 8 * cc : 8 * cc + 8, :])
            else:
                nc.scalar.copy(blk, c_all[:, 8 * cc : 8 * cc + 8, :])

        # transposes: 4 per [128,512] psum tile, then one copy out
        tp_count = [0]

        # xT: x_all[:, t, 128j:128j+128] -> xT[j][:, 128t..]
        for j in range(2):
            for tq in range(2):  # 4 tiles per copy
                pt = epp.tile([128, 512], F16, tag="pt", name="pt", bufs=2)
                for i in range(4):
                    t = 4 * tq + i
                    nc.tensor.transpose(
                        pt[:, 128 * i : 128 * i + 128],
                        xh[:, t, 128 * j : 128 * j + 128],
                        idh,
                    )
                dst = xT[j][:, 512 * tq : 512 * tq + 512]
                if tp_count[0] % 2 == 0:
                    nc.vector.tensor_copy(dst, pt)
                else:
                    nc.scalar.copy(dst, pt)
                tp_count[0] += 1
        for j in range(2):
            for tq in range(8):
                pt = epp.tile([128, 512], F16, tag="pt", name="pt", bufs=2)
                for i in range(4):
                    t = 4 * tq + i
                    nc.tensor.transpose(
                        pt[:, 128 * i : 128 * i + 128],
                        ch[:, t, 128 * j : 128 * j + 128],
                        idh,
                    )
                dst = cT[j][:, 512 * tq : 512 * tq + 512]
                if tp_count[0] % 2 == 0:
                    nc.vector.tensor_copy(dst, pt)
                else:
                    nc.scalar.copy(dst, pt)
                tp_count[0] += 1

        # ---- projections ----
        # qT[g] = (w_q[:, 128g:128g+128])^T @ xT ; kT[g] likewise from w_kv k-part
        for g in range(2):
            for nt in range(TL // 512):
                pq = epp.tile([128, 512], FP, tag="pj", name="pq", bufs=4)
                for kj in range(2):
                    nc.tensor.matmul(
                        pq,
                        lhsT=wqh[kj][:, 128 * g : 128 * g + 128],
                        rhs=xT[kj][:, 512 * nt : 512 * nt + 512],
                        start=(kj == 0),
                        stop=(kj == 1),
                    )
                dst = qT[g][:, 512 * nt : 512 * nt + 512]
                if nt % 2 == 0:
                    nc.vector.tensor_copy(dst, pq)
                else:
                    nc.scalar.copy(dst, pq)
            for nt in range(S // 512):
                pk = epp.tile([128, 512], FP, tag="pj", name="pk", bufs=4)
                for kj in range(2):
                    nc.tensor.matmul(
                        pk,
                        lhsT=wkh[kj][:, 128 * g : 128 * g + 128],
                        rhs=cT[kj][:, 512 * nt : 512 * nt + 512],
                        start=(kj == 0),
                        stop=(kj == 1),
                    )
                dst = kT[g][:, 512 * nt : 512 * nt + 512]
                if nt % 2 == 0:
                    nc.vector.tensor_copy(dst, pk)
                else:
                    nc.scalar.copy(dst, pk)

        # v: per chunk [128s, 256(h,dv)] -> vP strided; ones column via memset
        nc.vector.memset(vP[:, :, :, 32:33], 1.0)
        for c in range(NST):
            pv = epp.tile([128, D], FP, tag="pv", name="pv", bufs=2)
            for kj in range(2):
                nc.tensor.matmul(
                    pv,
                    lhsT=cT[kj][:, 128 * c : 128 * c + 128],
                    rhs=wvh[kj],
                    start=(kj == 0),
                    stop=(kj == 1),
                )
            dst = vP[:, c, :, 0:32]
            src = pv.rearrange("p (h w) -> p h w", h=H)
            if c % 2 == 0:
                nc.vector.tensor_copy(dst, src)
            else:
                nc.scalar.copy(dst, src)

        eps.__exit__(None, None, None)
        ea.__exit__(None, None, None)

        # ---------------- Phase B: attention ----------------
        phb = tc.tile_pool(name="slots", bufs=3, space="PSUM")
        psl = phb.__enter__()
        phb2 = tc.tile_pool(name="accs", bufs=2, space="PSUM")
        psa = phb2.__enter__()
        atp = ctx.enter_context(tc.tile_pool(name="atp", bufs=10))
        scfp = ctx.enter_context(tc.tile_pool(name="scfp", bufs=3))
        npool = ctx.enter_context(tc.tile_pool(name="npool", bufs=2))

        for g in range(2):
            for tp in range(2):
                accs = [
                    psa.tile([128, 512], FP, tag="acc", name=f"acc{g}{tp}{p}")
                    for p in range(2)
                ]
                for c in range(NST):
                    at_c = atp.tile([128, 4, 512], F16, tag="at", name="at")
                    for j in range(2):  # head-pair
                        slot = psl.tile([128, 1024], FP, tag="slot", name="slot")
                        for hh in range(2):
                            h = 2 * j + hh
                            nc.tensor.matmul(
                                slot[:, 512 * hh : 512 * hh + 512],
                                lhsT=kT[g][32 * h : 32 * h + 32, 128 * c : 128 * c + 128],
                                rhs=qT[g][32 * h : 32 * h + 32, 512 * tp : 512 * tp + 512],
                                start=True,
                                stop=True,
                                tile_position=(32 * h, 0),
                                skip_group_check=True,
                            )
                        at_sl = at_c[:, 2 * j : 2 * j + 2, :].rearrange("p a b -> p (a b)")
                        if j == 0:
                            # exact exp on ScalarE
                            nc.scalar.activation(
                                at_sl, slot, mybir.ActivationFunctionType.Exp, scale=SCALE
                            )
                        else:
                            # schraudolph exp on DVE: copy to fp16, then bit-trick
                            scf = scfp.tile([128, 1024], F16, tag="scf", name="scf")
                            nc.vector.tensor_copy(scf, slot)
                            nc.vector.tensor_scalar(
                                out=at_sl.bitcast(I16),
                                in0=scf,
                                scalar1=SCHR_A,
                                scalar2=SCHR_B,
                                op0=mybir.AluOpType.mult,
                                op1=mybir.AluOpType.add,
                            )
                    for p in range(2):
                        for hh in range(2):
                            h = 2 * p + hh
                            nc.tensor.matmul(
                                accs[p][64 * hh : 64 * hh + 33, :],
                                lhsT=vP[:, c, 4 * g + h, 0:33],
                                rhs=at_c[:, h, :],
                                start=(c == 0),
                                stop=(c == NST - 1),
                                tile_position=(0, 64 * hh),
                                skip_group_check=True,
                            )

                # normalize: denominators via DRAM broadcast roundtrip
                outU = npool.tile([128, 512], F16, tag="outU", name="outU")
                rden = npool.tile([128, 512], FP, tag="rden", name="rden")
                rcp = npool.tile([128, 512], FP, tag="rcp", name="rcp")
                for p in range(2):
                    for hh in range(2):
                        h = 2 * p + hh
                        nc.vector.tensor_copy(
                            outU[32 * h : 32 * h + 32, :],
                            accs[p][64 * hh : 64 * hh + 32, :],
                        )
                        dnt = npool.tile([1, 512], FP, tag=f"dnt{h}", name=f"dnt{h}")
                        nc.vector.tensor_copy(
                            dnt, accs[p][64 * hh + 32 : 64 * hh + 33, :]
                        )
                        nc.sync.dma_start(
                            out=dnscr[4 * g + h : 4 * g + h + 1, 512 * tp : 512 * tp + 512],
                            in_=dnt,
                        )
                for h in range(4):
                    nc.sync.dma_start(
                        out=rden[32 * h : 32 * h + 32, :],
                        in_=dnscr[
                            4 * g + h : 4 * g + h + 1, 512 * tp : 512 * tp + 512
                        ].partition_broadcast(32),
                    )
                nc.vector.reciprocal_approx_fast(rcp, rden)
                nc.vector.tensor_mul(
                    outN[g][:, 512 * tp : 512 * tp + 512], outU, rcp
                )

        phb2.__exit__(None, None, None)
        phb.__exit__(None, None, None)

        # ---------------- Phase C: output projection ----------------
        fps = ctx.enter_context(tc.tile_pool(name="fin_ps", bufs=3, space="PSUM"))
        fsb = ctx.enter_context(tc.tile_pool(name="fin_sb", bufs=3))
        for tt in range(TL // 128):
            fin = fps.tile([128, D], FP, tag="fin", name="fin")
            for g in range(2):
                nc.tensor.matmul(
                    fin,
                    lhsT=outN[g][:, 128 * tt : 128 * tt + 128],
                    rhs=woh[g],
                    start=(g == 0),
                    stop=(g == 1),
                )
            outs = fsb.tile([128, D], FP, tag="outs", name="outs")
            nc.vector.tensor_add(outs, fin, bias_c)
            nc.sync.dma_start(out=out_d[128 * tt : 128 * tt + 128, :], in_=outs)

    nc.compile()
    return nc


_NC = None


def kernel(**inputs):
    global _NC
    x = np.ascontiguousarray(inputs["x"], dtype=np.float32)
    context = np.ascontiguousarray(inputs["context"], dtype=np.float32)
    w_q = np.ascontiguousarray(inputs["w_q"], dtype=np.float32)
    w_kv = np.ascontiguousarray(inputs["w_kv"], dtype=np.float32)
    w_out = np.ascontiguousarray(inputs["w_out"], dtype=np.float32)
    b_out = np.ascontiguousarray(inputs["b_out"], dtype=np.float32).reshape(1, D)

    if _NC is None:
        _NC = build_bass()
    nc = _NC

    in_maps = []
    for c in range(8):
        b, half = c // 2, c % 2
        in_maps.append(
            {
                "ident": np.eye(128, dtype=np.float32),
                "x": np.ascontiguousarray(x[b, TL * half : TL * half + TL, :]),
                "context": np.ascontiguousarray(context[b]),
                "w_q": w_q,
                "w_kv": w_kv,
                "w_out": w_out,
                "b_out": b_out,
            }
        )
    res = run_bass_kernel_spmd(nc, in_maps, core_ids=list(range(8)))
    out = np.empty((B, T, D), dtype=np.float32)
    for c in range(8):
        b, half = c // 2, c % 2
        out[b, TL * half : TL * half + TL, :] = res.results[c]["out"]
    return out


if __name__ == "__main__":
    rng = np.random.default_rng(0)
    ins = {
        "x": rng.standard_normal((B, T, D), dtype=np.float32),
        "context": rng.standard_normal((B, S, D), dtype=np.float32),
        "w_q": rng.standard_normal((D, D), dtype=np.float32) * D**-0.5,
        "w_kv": rng.standard_normal((D, 2 * D), dtype=np.float32) * D**-0.5,
        "w_out": rng.standard_normal((D, D), dtype=np.float32) * D**-0.5,
        "b_out": rng.standard_normal((D,), dtype=np.float32) * 0.01,
    }
    out = kernel(**ins)
    print(out.shape, out.dtype, np.abs(out).mean())
